# revision 2
# baseline (speedup 1.0000x reference)
"""Bayesian linear layer (Monte-Carlo reparameterized GEMM) on 8 Trainium2 cores.

y[s,b,o] = sum_i x[b,i] * (w_mu[o,i] + exp(w_lsigma[o,i]) * r1[s,o,i]) + b_mu[o]
           + exp(b_lsigma[o]) * r2[s,o]

Decomposition:  y[s] = base + (x @ (E o r1[s])^T) + bias[s]
with base = x @ w_mu^T shared across all 64 samples (computed host-side in
f32 BLAS — 1.5% of the FLOPs) and the per-sample term scaled by
E = exp(w_lsigma) ~ 0.1, so it tolerates fp8: the host pre-quantizes
x^T -> e4m3 and (E o r1[s])^T -> e4m3 and the device runs the 64 sample
GEMMs as fp8 DoubleRow matmuls (0.5 cycles/row, 2 k-tiles per instruction).

Sharding: samples split across the 8 cores (8 samples/core); x^T and base^T
replicated.

Per-core device kernel (all in transposed orientation, partition = o):
  - xT fp8 [128,KT,B] and baseT fp16 [128,OT,B] resident in SBUF
  - per sample: stream r1T fp8 [128,KT,O] + biasT [128,OT] (double-buffered)
  - psum[ot,bj] [128,1024] (2 banks) accumulates 8 DoubleRow matmuls
  - evict y^T = psum + biasT[:,ot] (per-partition scalar) + baseT tile,
    fused in one DVE scalar_tensor_tensor for ~half the tiles, the rest on
    ACT (Identity + per-partition bias AP) followed by a Pool add
  - y^T out as fp16 on the two HWDGE queues (SP/ACT), host re-transposes
"""

import sys

if "/opt/trn_rl_repo" not in sys.path:
    sys.path.insert(0, "/opt/trn_rl_repo")

from contextlib import ExitStack

import ml_dtypes
import numpy as np

import concourse.bass as bass  # noqa: F401
import concourse.tile as tile
from concourse import bacc, mybir
from concourse.bass_utils import run_bass_kernel_spmd

P = 128
N_IN = 1024
N_OUT = 1024
BATCH = 4096
S = 64
NCORES = 8
SC = S // NCORES  # samples per core
KT = N_IN // P  # 8 k-tiles
OT = N_OUT // P  # 8 o-tiles
NB = 4  # b-blocks of 1024 per psum tile
BW = BATCH // NB  # 1024

F32 = mybir.dt.float32
F16 = mybir.dt.float16
F8 = mybir.dt.float8e4
NP_F8 = ml_dtypes.float8_e4m3fn

_CACHE = {}


def build_bass():
    nc = bacc.Bacc("TRN2", target_bir_lowering=False, debug=False)

    xT8 = nc.dram_tensor("xT8", [N_IN, BATCH], F8, kind="ExternalInput").ap()
    baseT = nc.dram_tensor("baseT", [N_OUT, BATCH], F16, kind="ExternalInput").ap()
    r1Ts = nc.dram_tensor("r1Ts", [SC, N_IN, N_OUT], F8, kind="ExternalInput").ap()
    biasTs = nc.dram_tensor("biasTs", [SC, N_OUT], F32, kind="ExternalInput").ap()
    y = nc.dram_tensor("y", [SC, N_OUT, BATCH], F16, kind="ExternalOutput").ap()

    DR = mybir.MatmulPerfMode.DoubleRow
    ADD = mybir.AluOpType.add

    with tile.TileContext(nc) as tc, ExitStack() as ctx:
        const = ctx.enter_context(tc.tile_pool(name="const", bufs=1))
        r1_pool = ctx.enter_context(tc.tile_pool(name="r1", bufs=2))
        bias_pool = ctx.enter_context(tc.tile_pool(name="bias", bufs=2))
        y_pool = ctx.enter_context(tc.tile_pool(name="yp", bufs=8))
        pm_pool = ctx.enter_context(tc.tile_pool(name="pm", bufs=4, space="PSUM"))

        xT8_sb = const.tile([P, KT, BATCH], F8)
        baseT_sb = const.tile([P, OT, BATCH], F16)

        def load_sample(s):
            r1t = r1_pool.tile([P, KT, N_OUT], F8, tag="r1", name=f"r1_{s}")
            nc.scalar.dma_start(
                r1t[:], r1Ts[s].rearrange("(k p) o -> p k o", p=P)
            )
            bt = bias_pool.tile([P, OT], F32, tag="bias", name=f"bias_{s}")
            nc.sync.dma_start(bt[:], biasTs[s].rearrange("(t p) -> p t", p=P))
            return r1t, bt

        # prologue: first b-chunk of xT8 + sample 0's r1/bias land first so the
        # PE can start; baseT tiles follow (gate only the evictions)
        nc.sync.dma_start(
            xT8_sb[:, :, 0:BW],
            xT8[:, 0:BW].rearrange("(k p) b -> p k b", p=P),
        )
        r1t, bt = load_sample(0)
        for bj in range(1, NB):
            nc.sync.dma_start(
                xT8_sb[:, :, bj * BW : (bj + 1) * BW],
                xT8[:, bj * BW : (bj + 1) * BW].rearrange("(k p) b -> p k b", p=P),
            )
        for o in range(OT):
            q = nc.scalar if o % 2 == 0 else nc.sync
            q.dma_start(baseT_sb[:, o, :], baseT[o * P : (o + 1) * P, :])

        for s in range(SC):
            nxt = load_sample(s + 1) if s + 1 < SC else None
            idx = 0
            for ot in range(OT):
                for bj in range(NB):
                    ps = pm_pool.tile([P, 2 * 512], F32, tag="pm")
                    for j in range(KT // 2):
                        for bc in range(2):
                            nc.tensor.matmul(
                                ps[:, bc * 512 : (bc + 1) * 512],
                                r1t[:, 2 * j : 2 * j + 2, ot * P : (ot + 1) * P],
                                xT8_sb[
                                    :,
                                    2 * j : 2 * j + 2,
                                    bj * BW + bc * 512 : bj * BW + (bc + 1) * 512,
                                ],
                                start=(j == 0),
                                stop=(j == KT // 2 - 1),
                                perf_mode=DR,
                            )
                    yt = y_pool.tile([P, BW], F16, tag="y")
                    bsl = slice(bj * BW, (bj + 1) * BW)
                    if idx % 9 in (0, 2, 4, 6):
                        # fused psum + bias (per-partition scalar) + base
                        nc.vector.scalar_tensor_tensor(
                            yt[:],
                            ps[:],
                            bt[:, ot : ot + 1],
                            baseT_sb[:, ot, bsl],
                            ADD,
                            ADD,
                        )
                    else:
                        nc.scalar.add(yt[:], ps[:], add=bt[:, ot : ot + 1])
                        nc.gpsimd.tensor_add(yt[:], yt[:], baseT_sb[:, ot, bsl])
                    q = nc.sync if idx % 2 == 0 else nc.scalar
                    q.dma_start(y[s, ot * P : (ot + 1) * P, bsl], yt[:])
                    idx += 1
            if nxt is not None:
                r1t, bt = nxt

    nc.compile()
    return nc


def _get_nc():
    if "nc" not in _CACHE:
        _CACHE["nc"] = build_bass()
    return _CACHE["nc"]


def _prep(x, w_mu, w_lsigma, b_mu, b_lsigma, r1, r2):
    """Host-side marshalling.

    Returns (consts, r18, biasT):
      consts: shared per-core inputs {xT8, baseT}
      r18:    [S, N_IN, N_OUT] e4m3 of (exp(w_lsigma) o r1)^T
      biasT:  [S, N_OUT] f32 of b_mu + exp(b_lsigma)*r2
    """
    biasT = (b_mu[None, :] + np.exp(b_lsigma)[None, :] * r2).astype(np.float32)
    base = x @ w_mu.T  # f32 BLAS, shared across samples
    baseT = np.ascontiguousarray(base.T).astype(np.float16)
    xT8 = np.ascontiguousarray(x.T).astype(NP_F8)
    if np.all(w_lsigma == w_lsigma.flat[0]):
        c = np.float32(np.exp(w_lsigma.flat[0]))
        r1e = r1 * c
    else:
        r1e = r1 * np.exp(w_lsigma)[None, :, :]
    # [S, O, I] -> [S, I, O] quantized; astype on the transposed view does
    # the strided read + contiguous write in one pass
    r18 = r1e.transpose(0, 2, 1).astype(NP_F8)
    return {"xT8": xT8, "baseT": baseT}, r18, biasT


def _in_maps(consts, r18, biasT):
    maps = []
    for c in range(NCORES):
        sl = slice(c * SC, (c + 1) * SC)
        maps.append(
            dict(
                consts,
                r1Ts=np.ascontiguousarray(r18[sl]),
                biasTs=np.ascontiguousarray(biasT[sl]),
            )
        )
    return maps


def _assemble(res):
    out = np.empty((S, BATCH, N_OUT), np.float32)
    for c in range(NCORES):
        yT = res.results[c]["y"]  # [SC, N_OUT, BATCH] f16
        out[c * SC : (c + 1) * SC] = yT.transpose(0, 2, 1).astype(np.float32)
    return out


def kernel(x, w_mu, w_lsigma, b_mu, b_lsigma, r1, r2, N_samples):
    x = np.asarray(x, dtype=np.float32)
    w_mu = np.asarray(w_mu, dtype=np.float32)
    w_lsigma = np.asarray(w_lsigma, dtype=np.float32)
    b_mu = np.asarray(b_mu, dtype=np.float32)
    b_lsigma = np.asarray(b_lsigma, dtype=np.float32)
    r1 = np.asarray(r1, dtype=np.float32)
    r2 = np.asarray(r2, dtype=np.float32)
    assert x.shape == (BATCH, N_IN) and r1.shape == (S, N_OUT, N_IN)

    consts, r18, biasT = _prep(x, w_mu, w_lsigma, b_mu, b_lsigma, r1, r2)
    nc = _get_nc()
    res = run_bass_kernel_spmd(nc, _in_maps(consts, r18, biasT),
                               core_ids=list(range(NCORES)))
    return _assemble(res)


# revision 5
# speedup vs baseline: 1.0114x; 1.0114x over previous
"""Bayesian linear layer (Monte-Carlo reparameterized GEMM) on 8 Trainium2 cores.

y[s,b,o] = sum_i x[b,i] * (w_mu[o,i] + exp(w_lsigma[o,i]) * r1[s,o,i]) + b_mu[o]
           + exp(b_lsigma[o]) * r2[s,o]

Decomposition:  y[s] = base + (x @ (E o r1[s])^T) + bias[s]
with base = x @ w_mu^T shared across all 64 samples (computed host-side in
f32 BLAS — 1.5% of the FLOPs) and the per-sample term scaled by
E = exp(w_lsigma) ~ 0.1, so it tolerates fp8: the host pre-quantizes
x^T -> e4m3 and (E o r1[s])^T -> e4m3 and the device runs the 64 sample
GEMMs as fp8 DoubleRow matmuls (0.5 cycles/row, 2 k-tiles per instruction).

Sharding: samples split across the 8 cores (8 samples/core); x^T and base^T
replicated.

Per-core device kernel (all in transposed orientation, partition = o):
  - xT fp8 [128,KT,B] and baseT fp16 [128,OT,B] resident in SBUF
  - per sample: stream r1T fp8 [128,KT,O] + biasT [128,OT] (double-buffered)
  - psum[ot,bj] [128,1024] (2 banks) accumulates 8 DoubleRow matmuls
  - evict y^T = psum + biasT[:,ot] (per-partition scalar) + baseT tile,
    fused in one DVE scalar_tensor_tensor for ~half the tiles, the rest on
    ACT (Identity + per-partition bias AP) followed by a Pool add
  - y^T out as fp16 on the two HWDGE queues (SP/ACT), host re-transposes
"""

import sys

if "/opt/trn_rl_repo" not in sys.path:
    sys.path.insert(0, "/opt/trn_rl_repo")

from contextlib import ExitStack

import ml_dtypes
import numpy as np

import concourse.bass as bass  # noqa: F401
import concourse.tile as tile
from concourse import bacc, mybir
from concourse.bass_utils import run_bass_kernel_spmd

P = 128
N_IN = 1024
N_OUT = 1024
BATCH = 4096
S = 64
NCORES = 8
SC = S // NCORES  # samples per core
KT = N_IN // P  # 8 k-tiles
OT = N_OUT // P  # 8 o-tiles
NB = 4  # b-blocks of 1024 per psum tile
BW = BATCH // NB  # 1024

F32 = mybir.dt.float32
F16 = mybir.dt.float16
F8 = mybir.dt.float8e4
NP_F8 = ml_dtypes.float8_e4m3fn

_CACHE = {}


def build_bass():
    nc = bacc.Bacc("TRN2", target_bir_lowering=False, debug=False)

    xT8 = nc.dram_tensor("xT8", [N_IN, BATCH], F8, kind="ExternalInput").ap()
    baseT = nc.dram_tensor("baseT", [N_OUT, BATCH], F16, kind="ExternalInput").ap()
    r1Ts = nc.dram_tensor("r1Ts", [SC, N_IN, N_OUT], F8, kind="ExternalInput").ap()
    biasTs = nc.dram_tensor("biasTs", [SC, N_OUT], F32, kind="ExternalInput").ap()
    y = nc.dram_tensor("y", [SC, N_OUT, BATCH], F16, kind="ExternalOutput").ap()

    DR = mybir.MatmulPerfMode.DoubleRow
    ADD = mybir.AluOpType.add

    with tile.TileContext(nc) as tc, ExitStack() as ctx:
        const = ctx.enter_context(tc.tile_pool(name="const", bufs=1))
        r1_pool = ctx.enter_context(tc.tile_pool(name="r1", bufs=2))
        bias_pool = ctx.enter_context(tc.tile_pool(name="bias", bufs=2))
        y_pool = ctx.enter_context(tc.tile_pool(name="yp", bufs=8))
        pm_pool = ctx.enter_context(tc.tile_pool(name="pm", bufs=4, space="PSUM"))

        xT8_sb = const.tile([P, KT, BATCH], F8)
        baseT_sb = const.tile([P, OT, BATCH], F16)

        def load_sample(s, q=None):
            r1t = r1_pool.tile([P, KT, N_OUT], F8, tag="r1", name=f"r1_{s}")
            (q or nc.scalar).dma_start(
                r1t[:], r1Ts[s].rearrange("(k p) o -> p k o", p=P)
            )
            bt = bias_pool.tile([P, OT], F32, tag="bias", name=f"bias_{s}")
            nc.sync.dma_start(bt[:], biasTs[s].rearrange("(t p) -> p t", p=P))
            return r1t, bt

        def load_xchunk(q, bj):
            q.dma_start(
                xT8_sb[:, :, bj * BW : (bj + 1) * BW],
                xT8[:, bj * BW : (bj + 1) * BW].rearrange("(k p) b -> p k b", p=P),
            )

        def load_base(q, o):
            q.dma_start(baseT_sb[:, o, :], baseT[o * P : (o + 1) * P, :])

        # prologue, consumption-ordered across the 3 DMA queues: sample 0
        # sweeps bj-outer/ot-inner, so it needs xT8 chunk bj at ~13.6*bj us
        # and baseT[ot] at ~1.7*ot us into the first sweep.
        load_xchunk(nc.sync, 0)
        r1t, bt = load_sample(0, q=nc.scalar)
        load_base(nc.sync, 0)
        load_base(nc.scalar, 1)
        load_base(nc.sync, 2)
        load_base(nc.scalar, 3)
        load_base(nc.sync, 4)
        load_base(nc.scalar, 5)
        load_xchunk(nc.sync, 1)
        load_base(nc.scalar, 6)
        load_xchunk(nc.sync, 2)
        load_base(nc.scalar, 7)
        load_xchunk(nc.sync, 3)

        for s in range(SC):
            nxt = load_sample(s + 1) if s + 1 < SC else None
            idx = 0
            for bj in range(NB):
                for ot in range(OT):
                    ps = pm_pool.tile([P, 2 * 512], F32, tag="pm")
                    for j in range(KT // 2):
                        for bc in range(2):
                            nc.tensor.matmul(
                                ps[:, bc * 512 : (bc + 1) * 512],
                                r1t[:, 2 * j : 2 * j + 2, ot * P : (ot + 1) * P],
                                xT8_sb[
                                    :,
                                    2 * j : 2 * j + 2,
                                    bj * BW + bc * 512 : bj * BW + (bc + 1) * 512,
                                ],
                                start=(j == 0),
                                stop=(j == KT // 2 - 1),
                                perf_mode=DR,
                            )
                    yt = y_pool.tile([P, BW], F16, tag="y")
                    bsl = slice(bj * BW, (bj + 1) * BW)
                    # fused psum + bias (per-partition scalar) + base on DVE
                    nc.vector.scalar_tensor_tensor(
                        yt[:],
                        ps[:],
                        bt[:, ot : ot + 1],
                        baseT_sb[:, ot, bsl],
                        ADD,
                        ADD,
                    )
                    q = nc.sync if idx % 2 == 0 else nc.scalar
                    q.dma_start(y[s, ot * P : (ot + 1) * P, bsl], yt[:])
                    idx += 1
            if nxt is not None:
                r1t, bt = nxt

    nc.compile()
    return nc


def _get_nc():
    if "nc" not in _CACHE:
        _CACHE["nc"] = build_bass()
    return _CACHE["nc"]


def _prep(x, w_mu, w_lsigma, b_mu, b_lsigma, r1, r2):
    """Host-side marshalling.

    Returns (consts, r18, biasT):
      consts: shared per-core inputs {xT8, baseT}
      r18:    [S, N_IN, N_OUT] e4m3 of (exp(w_lsigma) o r1)^T
      biasT:  [S, N_OUT] f32 of b_mu + exp(b_lsigma)*r2
    """
    biasT = (b_mu[None, :] + np.exp(b_lsigma)[None, :] * r2).astype(np.float32)
    base = x @ w_mu.T  # f32 BLAS, shared across samples
    baseT = np.ascontiguousarray(base.T).astype(np.float16)
    xT8 = np.ascontiguousarray(x.T).astype(NP_F8)
    if np.all(w_lsigma == w_lsigma.flat[0]):
        c = np.float32(np.exp(w_lsigma.flat[0]))
        r1e = r1 * c
    else:
        r1e = r1 * np.exp(w_lsigma)[None, :, :]
    # [S, O, I] -> [S, I, O] quantized; astype on the transposed view does
    # the strided read + contiguous write in one pass
    r18 = r1e.transpose(0, 2, 1).astype(NP_F8)
    return {"xT8": xT8, "baseT": baseT}, r18, biasT


def _in_maps(consts, r18, biasT):
    maps = []
    for c in range(NCORES):
        sl = slice(c * SC, (c + 1) * SC)
        maps.append(
            dict(
                consts,
                r1Ts=np.ascontiguousarray(r18[sl]),
                biasTs=np.ascontiguousarray(biasT[sl]),
            )
        )
    return maps


def _assemble(res):
    out = np.empty((S, BATCH, N_OUT), np.float32)
    for c in range(NCORES):
        yT = res.results[c]["y"]  # [SC, N_OUT, BATCH] f16
        out[c * SC : (c + 1) * SC] = yT.transpose(0, 2, 1).astype(np.float32)
    return out


def kernel(x, w_mu, w_lsigma, b_mu, b_lsigma, r1, r2, N_samples):
    x = np.asarray(x, dtype=np.float32)
    w_mu = np.asarray(w_mu, dtype=np.float32)
    w_lsigma = np.asarray(w_lsigma, dtype=np.float32)
    b_mu = np.asarray(b_mu, dtype=np.float32)
    b_lsigma = np.asarray(b_lsigma, dtype=np.float32)
    r1 = np.asarray(r1, dtype=np.float32)
    r2 = np.asarray(r2, dtype=np.float32)
    assert x.shape == (BATCH, N_IN) and r1.shape == (S, N_OUT, N_IN)

    consts, r18, biasT = _prep(x, w_mu, w_lsigma, b_mu, b_lsigma, r1, r2)
    nc = _get_nc()
    res = run_bass_kernel_spmd(nc, _in_maps(consts, r18, biasT),
                               core_ids=list(range(NCORES)))
    return _assemble(res)


# revision 9
# speedup vs baseline: 1.0161x; 1.0047x over previous
"""Bayesian linear layer (Monte-Carlo reparameterized GEMM) on 8 Trainium2 cores.

y[s,b,o] = sum_i x[b,i] * (w_mu[o,i] + exp(w_lsigma[o,i]) * r1[s,o,i]) + b_mu[o]
           + exp(b_lsigma[o]) * r2[s,o]

Decomposition:  y[s] = base + (x @ (E o r1[s])^T) + bias[s]
with base = x @ w_mu^T shared across all 64 samples (computed host-side in
f32 BLAS — 1.5% of the FLOPs) and the per-sample term scaled by
E = exp(w_lsigma) ~ 0.1, so it tolerates fp8: the host pre-quantizes
x^T -> e4m3 and (E o r1[s])^T -> e4m3 and the device runs the 64 sample
GEMMs as fp8 DoubleRow matmuls (2 rows/cycle, 2 k-tiles per instruction,
157 TF/s — the fp8 peak; measured 216ns per [256x128x512] matmul vs the
213.3ns floor).

Sharding: samples split across the 8 cores (8 samples/core); x^T and base^T
replicated. All device inputs are pre-tiled on host into per-partition-
contiguous layouts so every DMA moves >=4KB runs (1KB-fragmented gathers
run at ~100GB/s vs ~330GB/s for contiguous).

Per-core device kernel (transposed orientation, partition = o):
  - xT fp8 [128][NB][KT*BW] and baseT fp16 [128][OT][B] resident in SBUF
  - per sample: stream r1T fp8 [128][KT*O] + biasT (double-buffered)
  - psum[ot,bj] [128,1024] (2 banks) accumulates 8 DoubleRow matmuls
  - evict y^T = psum + biasT[:,ot] (per-partition scalar) + baseT tile,
    fused in one DVE scalar_tensor_tensor; final sweep splits DVE/ACT
  - y^T out as fp16 on the two HWDGE queues (SP/ACT), host re-transposes
"""

import sys

if "/opt/trn_rl_repo" not in sys.path:
    sys.path.insert(0, "/opt/trn_rl_repo")

from contextlib import ExitStack

import ml_dtypes
import numpy as np

import concourse.bass as bass  # noqa: F401
import concourse.tile as tile
from concourse import bacc, mybir
from concourse.bass_utils import run_bass_kernel_spmd

P = 128
N_IN = 1024
N_OUT = 1024
BATCH = 4096
S = 64
NCORES = 8
SC = S // NCORES  # samples per core
KT = N_IN // P  # 8 k-tiles
OT = N_OUT // P  # 8 o-tiles
NB = 4  # b-blocks of 1024 per psum tile
BW = BATCH // NB  # 1024

F32 = mybir.dt.float32
F16 = mybir.dt.float16
F8 = mybir.dt.float8e4
NP_F8 = ml_dtypes.float8_e4m3fn

_CACHE = {}


def build_bass():
    nc = bacc.Bacc("TRN2", target_bir_lowering=False, debug=False)

    # pre-tiled HBM layouts: per-partition contiguous runs
    xT8 = nc.dram_tensor("xT8", [NB, P, KT * BW], F8, kind="ExternalInput").ap()
    baseT = nc.dram_tensor("baseT", [OT, P, BATCH], F16, kind="ExternalInput").ap()
    r1Ts = nc.dram_tensor("r1Ts", [SC, P, KT * N_OUT], F8, kind="ExternalInput").ap()
    biasTs = nc.dram_tensor("biasTs", [SC, P, OT], F32, kind="ExternalInput").ap()
    y = nc.dram_tensor("y", [SC, N_OUT, BATCH], F16, kind="ExternalOutput").ap()

    DR = mybir.MatmulPerfMode.DoubleRow
    ADD = mybir.AluOpType.add

    with tile.TileContext(nc) as tc, ExitStack() as ctx:
        const = ctx.enter_context(tc.tile_pool(name="const", bufs=1))
        r1_pool = ctx.enter_context(tc.tile_pool(name="r1", bufs=2))
        bias_pool = ctx.enter_context(tc.tile_pool(name="bias", bufs=2))
        y_pool = ctx.enter_context(tc.tile_pool(name="yp", bufs=8))
        pm_pool = ctx.enter_context(tc.tile_pool(name="pm", bufs=4, space="PSUM"))

        # b-block-major so each chunk DMA lands per-partition contiguous
        xT8_sb = const.tile([P, NB, KT, BW], F8)
        baseT_sb = const.tile([P, OT, BATCH], F16)

        # PE clock warmup: the tensor engine ramps 0.65 -> 1.2 -> 2.4 GHz
        # over ~3us of sustained work; stream dummy DoubleRow matmuls while
        # the prologue DMAs fly so real matmuls start at full clock.
        dum = const.tile([P, 2, 512], F8)
        nc.vector.memset(dum[:], 0)
        wps = pm_pool.tile([P, 2 * 512], F32, tag="pm", name="warm")
        for _ in range(6):
            nc.tensor.matmul(
                wps[:, 0:512], dum[:, :, 0:128], dum[:], perf_mode=DR
            )

        def load_sample(s):
            r1t = r1_pool.tile([P, KT, N_OUT], F8, tag="r1", name=f"r1_{s}")
            nc.scalar.dma_start(r1t[:], r1Ts[s].rearrange("p (k o) -> p k o", k=KT))
            bt = bias_pool.tile([P, OT], F32, tag="bias", name=f"bias_{s}")
            nc.sync.dma_start(bt[:], biasTs[s])
            return r1t, bt

        def load_xchunk(q, bj):
            q.dma_start(
                xT8_sb[:, bj, :, :],
                xT8[bj].rearrange("p (k b) -> p k b", k=KT),
            )

        def load_base(q, o, h):
            hw = BATCH // 2
            q.dma_start(
                baseT_sb[:, o, h * hw : (h + 1) * hw],
                baseT[o, :, h * hw : (h + 1) * hw],
            )

        # prologue, consumption-ordered: gating transfers (xT8 chunk 0,
        # sample-0 r1T) first on their queues; baseT halves follow in
        # first-sweep eviction order; later xT8 chunks interleaved when
        # they are needed (~13.6us per b-block sweep).
        load_xchunk(nc.sync, 0)
        r1t, bt = load_sample(0)
        for o in (0, 2, 4):
            load_base(nc.sync, o, 0)
            load_base(nc.sync, o, 1)
        for o in (1, 3, 5):
            load_base(nc.scalar, o, 0)
            load_base(nc.scalar, o, 1)
        load_xchunk(nc.sync, 1)
        load_base(nc.sync, 6, 0)
        load_base(nc.sync, 6, 1)
        load_base(nc.scalar, 7, 0)
        load_base(nc.scalar, 7, 1)
        load_xchunk(nc.scalar, 2)
        load_xchunk(nc.scalar, 3)

        for s in range(SC):
            nxt = load_sample(s + 1) if s + 1 < SC else None
            idx = 0
            for bj in range(NB):
                for ot in range(OT):
                    ps = pm_pool.tile([P, 2 * 512], F32, tag="pm")
                    for j in range(KT // 2):
                        for bc in range(2):
                            nc.tensor.matmul(
                                ps[:, bc * 512 : (bc + 1) * 512],
                                r1t[:, 2 * j : 2 * j + 2, ot * P : (ot + 1) * P],
                                xT8_sb[
                                    :, bj, 2 * j : 2 * j + 2,
                                    bc * 512 : (bc + 1) * 512,
                                ],
                                start=(j == 0),
                                stop=(j == KT // 2 - 1),
                                perf_mode=DR,
                            )
                    yt = y_pool.tile([P, BW], F16, tag="y")
                    bsl = slice(bj * BW, (bj + 1) * BW)
                    if s == SC - 1 and bj == NB - 1 and ot % 2 == 1:
                        # final sweep: split evictions across ACT+DVE so the
                        # tail drains in parallel
                        nc.scalar.add(yt[:], ps[:], add=bt[:, ot : ot + 1])
                        nc.vector.tensor_add(yt[:], yt[:], baseT_sb[:, ot, bsl])
                    else:
                        # fused psum + bias (per-partition scalar) + base
                        nc.vector.scalar_tensor_tensor(
                            yt[:],
                            ps[:],
                            bt[:, ot : ot + 1],
                            baseT_sb[:, ot, bsl],
                            ADD,
                            ADD,
                        )
                    q = nc.sync if idx % 2 == 0 else nc.scalar
                    q.dma_start(y[s, ot * P : (ot + 1) * P, bsl], yt[:])
                    idx += 1
            if nxt is not None:
                r1t, bt = nxt

    nc.compile()
    return nc


def _get_nc():
    if "nc" not in _CACHE:
        _CACHE["nc"] = build_bass()
    return _CACHE["nc"]


def _prep(x, w_mu, w_lsigma, b_mu, b_lsigma, r1, r2):
    """Host-side marshalling into pre-tiled device layouts.

    Returns (consts, r1dev, biasdev):
      consts:  shared per-core inputs {xT8 [NB,P,KT*BW], baseT [OT,P,B]}
      r1dev:   [S, P, KT*N_OUT] e4m3 of (exp(w_lsigma) o r1)^T, pre-tiled
      biasdev: [S, P, OT] f32 of b_mu + exp(b_lsigma)*r2, pre-tiled
    """
    biasT = (b_mu[None, :] + np.exp(b_lsigma)[None, :] * r2).astype(np.float32)
    # bias[s, o] with o = t*P + p -> [S, P, OT]
    biasdev = np.ascontiguousarray(biasT.reshape(S, OT, P).transpose(0, 2, 1))
    base = x @ w_mu.T  # f32 BLAS, shared across samples
    # base[b, o] -> baseT[ot, p, b] with o = ot*P + p
    basedev = np.ascontiguousarray(
        base.T.astype(np.float16).reshape(OT, P, BATCH)
    )
    x8 = np.ascontiguousarray(x.T).astype(NP_F8)  # [N_IN, BATCH]
    # [kt*P+p, bj*BW+b] -> [bj, p, kt*BW+b]
    xdev = np.ascontiguousarray(
        x8.reshape(KT, P, NB, BW).transpose(2, 1, 0, 3).reshape(NB, P, KT * BW)
    )
    if np.all(w_lsigma == w_lsigma.flat[0]):
        r1e = r1 * np.float32(np.exp(w_lsigma.flat[0]))
    else:
        r1e = r1 * np.exp(w_lsigma)[None, :, :]
    # [S, O, I] -> [S, I, O] quantized (one strided pass), then pre-tile
    # i = kt*P + p -> [S, P, KT*O]
    r18 = r1e.transpose(0, 2, 1).astype(NP_F8)
    r1dev = np.ascontiguousarray(
        r18.reshape(S, KT, P, N_OUT).transpose(0, 2, 1, 3).reshape(S, P, KT * N_OUT)
    )
    return {"xT8": xdev, "baseT": basedev}, r1dev, biasdev


def _in_maps(consts, r1dev, biasdev):
    maps = []
    for c in range(NCORES):
        sl = slice(c * SC, (c + 1) * SC)
        maps.append(
            dict(
                consts,
                r1Ts=np.ascontiguousarray(r1dev[sl]),
                biasTs=np.ascontiguousarray(biasdev[sl]),
            )
        )
    return maps


def _assemble(res):
    out = np.empty((S, BATCH, N_OUT), np.float32)
    for c in range(NCORES):
        yT = res.results[c]["y"]  # [SC, N_OUT, BATCH] f16
        out[c * SC : (c + 1) * SC] = yT.transpose(0, 2, 1).astype(np.float32)
    return out


def kernel(x, w_mu, w_lsigma, b_mu, b_lsigma, r1, r2, N_samples):
    x = np.asarray(x, dtype=np.float32)
    w_mu = np.asarray(w_mu, dtype=np.float32)
    w_lsigma = np.asarray(w_lsigma, dtype=np.float32)
    b_mu = np.asarray(b_mu, dtype=np.float32)
    b_lsigma = np.asarray(b_lsigma, dtype=np.float32)
    r1 = np.asarray(r1, dtype=np.float32)
    r2 = np.asarray(r2, dtype=np.float32)
    assert x.shape == (BATCH, N_IN) and r1.shape == (S, N_OUT, N_IN)

    consts, r1dev, biasdev = _prep(x, w_mu, w_lsigma, b_mu, b_lsigma, r1, r2)
    nc = _get_nc()
    res = run_bass_kernel_spmd(nc, _in_maps(consts, r1dev, biasdev),
                               core_ids=list(range(NCORES)))
    return _assemble(res)
